# revision 1
# baseline (speedup 1.0000x reference)
"""Trainium2 Bass kernel for nn_LSTMModel (B=2048, T=512, I=1, H=64, O=1).

Strategy: pure data parallel over 8 NeuronCores (256 batch rows each).
Within a core, 2 independent batch chains of 128 run the T=512 recurrence.

Layout is gate-major: gates live as [4H, B] (gates on partitions, batch on
the free dim), so the hidden state h stays [H, B] and feeds the next
matmul's moving operand directly -- no transposes anywhere.

Math tricks (weights pre-scaled on host):
  - store h~ = h/2, c~ = c/2
  - g-gate preactivation is doubled so sigmoid(2a) serves all 4 gates in a
    single ACT op per step: tanh(a) = 2*sigmoid(2a) - 1
  - c~ = f*c~ + (g^-0.5)*i^          (one fused scalar_tensor_tensor + 2 TT)
  - h~ = 0.5*tanh(2c~)*o^            (one ACT tanh + one fused STT)
  - x_t's rank-1 contribution and the biases ride inside the recurrent
    matmul as two extra K rows (x row via tiny prefetched DMAs, ones row).
"""

import numpy as np

B, T, I, H, O = 2048, 512, 1, 64, 1
NCORES = 8
BC = B // NCORES          # 256 batch rows per core
NCHAINS = 4
BCH = BC // NCHAINS       # 64 batch rows per chain
NB = 4                    # state ring buffers per chain
K = H + 2                 # h rows + x row + ones row

_CACHE = {}

R_ENG = lambda nc: nc.gpsimd   # engine for r = f*c
A_ENG = lambda nc: nc.vector   # engine for c = q + r


def _build_program(T=T):
    import concourse.bacc as bacc
    import concourse.tile as tile
    from concourse import mybir

    f32 = mybir.dt.float32
    AF = mybir.ActivationFunctionType
    OP = mybir.AluOpType

    nc = bacc.Bacc("TRN2", target_bir_lowering=False, debug=False)

    wm_d = nc.dram_tensor("wm", (K, 256), f32, kind="ExternalInput").ap()
    wout_d = nc.dram_tensor("wout", (K, 1), f32, kind="ExternalInput").ap()
    xt_d = [
        nc.dram_tensor(f"xt{ch}", (T, BCH), f32, kind="ExternalInput").ap()
        for ch in range(NCHAINS)
    ]
    y_d = nc.dram_tensor("y", (NCHAINS, BCH), f32, kind="ExternalOutput").ap()

    with tile.TileContext(nc) as tc:
        with (
            tc.tile_pool(name="consts", bufs=1) as wpool,
            tc.tile_pool(name="state", bufs=1) as spool,
            tc.tile_pool(name="gates", bufs=3) as gpool,
            tc.tile_pool(name="tmp", bufs=3) as tpool,
            tc.tile_pool(name="psum", bufs=max(1, 6 // NCHAINS), space="PSUM") as pspool,
            tc.tile_pool(name="opsum", bufs=1, space="PSUM") as opspool,
        ):
            wm = wpool.tile([K, 256], f32, tag="wm")
            wo = wpool.tile([K, 1], f32, tag="wo")
            nc.sync.dma_start(wm[:], wm_d[:])
            nc.sync.dma_start(wo[:], wout_d[:])

            cst = []
            st = []
            for ch in range(NCHAINS):
                c = spool.tile([H, BCH], f32, tag=f"c{ch}")
                nc.vector.memset(c[:], 0.0)
                cst.append(c)
                bufs = []
                for b in range(NB):
                    s = spool.tile([K, BCH], f32, tag=f"s{ch}_{b}")
                    nc.vector.memset(s[0:H, :], 0.0)
                    nc.vector.memset(s[H : H + 1, :], 1.0)
                    bufs.append(s)
                st.append(bufs)

            for t in range(T):
                for ch in range(NCHAINS):
                    sb = st[ch][t % NB]
                    c = cst[ch]
                    # x_t row (prefetched; ~NB steps of slack)
                    nc.sync.dma_start(sb[H + 1 : K, :], xt_d[ch][t : t + 1, :])
                    # 4x M=64 matmuls, one per gate, all at partitions 0:64
                    # psum cols: [g | f | i | o] x BCH batch each
                    ps = pspool.tile([H, 4 * BCH], f32, tag=f"ps{ch}")
                    for k in range(4):
                        nc.tensor.matmul(
                            ps[:, k * BCH : (k + 1) * BCH],
                            wm[:, k * H : (k + 1) * H],
                            sb[:],
                            start=True,
                            stop=True,
                        )
                    # one sigmoid for all 4 gates
                    g = gpool.tile([H, 4 * BCH], f32, tag=f"g{ch}")
                    nc.scalar.activation(g[:], ps[:], AF.Sigmoid)
                    gh = g[:, 0:BCH]
                    fh = g[:, BCH : 2 * BCH]
                    ih = g[:, 2 * BCH : 3 * BCH]
                    oh = g[:, 3 * BCH : 4 * BCH]
                    # q~ = (g^ - 0.5) * i^
                    q = tpool.tile([H, BCH], f32, tag=f"q{ch}")
                    nc.vector.scalar_tensor_tensor(
                        q[:], gh, 0.5, ih, OP.subtract, OP.mult
                    )
                    # r = f^ * c~
                    r = tpool.tile([H, BCH], f32, tag=f"r{ch}")
                    R_ENG(nc).tensor_mul(r[:], fh, c[:])
                    # c~ = q~ + r   (in place)
                    A_ENG(nc).tensor_add(c[:], q[:], r[:])
                    # tc = tanh(2 c~) = tanh(c)
                    tch = tpool.tile([H, BCH], f32, tag=f"t{ch}")
                    nc.scalar.activation(tch[:], c[:], AF.Tanh, scale=2.0)
                    # h~ = (tc * 0.5) * o^  -> next state buffer's h rows
                    nxt = st[ch][(t + 1) % NB]
                    nc.vector.scalar_tensor_tensor(
                        nxt[0:H, :], tch[:], 0.5, oh, OP.mult, OP.mult
                    )

            for ch in range(NCHAINS):
                pso = opspool.tile([1, BCH], f32, tag="po")
                nc.tensor.matmul(
                    pso[:], wo[:], st[ch][T % NB][:], start=True, stop=True
                )
                yt = tpool.tile([1, BCH], f32, tag=f"y{ch}")
                nc.vector.tensor_copy(yt[:], pso[:])
                nc.sync.dma_start(y_d[ch : ch + 1, :], yt[:])

    nc.compile()
    return nc


def _prep_weights(w_ih, w_hh, b_ih, b_hh, w_lin, b_lin):
    """Host-side pre-scaled stationary operands ([K, M] = lhsT layout)."""
    w_hh = np.asarray(w_hh, np.float32)
    w_ih = np.asarray(w_ih, np.float32)
    bias = np.asarray(b_ih, np.float32) + np.asarray(b_hh, np.float32)
    # per-gate-row scale: 1 for i,f,o; 2 for g (sigmoid(2a) trick)
    s = np.ones((4 * H, 1), np.float32)
    s[2 * H : 3 * H] = 2.0
    # state row layout: [h (0:64); ones (64); x (65)]
    waug = np.concatenate(
        [2.0 * s * w_hh, s * bias[:, None], s * w_ih[:, :1]], axis=1
    )  # [4H, K]
    i_r = slice(0, H)
    f_r = slice(H, 2 * H)
    g_r = slice(2 * H, 3 * H)
    o_r = slice(3 * H, 4 * H)
    # lhsT col-blocks in [g | f | i | o] order
    wmat = np.ascontiguousarray(
        np.concatenate([waug[g_r], waug[f_r], waug[i_r], waug[o_r]], axis=0).T
    )  # [K, 256]
    wout = np.zeros((K, 1), np.float32)
    wout[0:H, 0] = 2.0 * np.asarray(w_lin, np.float32)[0]
    wout[H, 0] = float(np.asarray(b_lin, np.float32)[0])
    return wmat, wout


def kernel(x, w_ih, w_hh, b_ih, b_hh, w_lin, b_lin):
    from concourse import bass_utils

    if "nc" not in _CACHE:
        _CACHE["nc"] = _build_program()
    nc = _CACHE["nc"]

    wmat, wout = _prep_weights(w_ih, w_hh, b_ih, b_hh, w_lin, b_lin)

    x = np.asarray(x, np.float32).reshape(B, T)  # I == 1
    in_maps = []
    for core in range(NCORES):
        xc = x[core * BC : (core + 1) * BC]  # [BC, T]
        m = {"wm": wmat, "wout": wout}
        for ch in range(NCHAINS):
            xch = xc[ch * BCH : (ch + 1) * BCH]  # [BCH, T]
            m[f"xt{ch}"] = np.ascontiguousarray(xch.T)  # [T, BCH]
        in_maps.append(m)

    res = bass_utils.run_bass_kernel_spmd(
        nc, in_maps, core_ids=list(range(NCORES))
    )
    out = np.concatenate(
        [r["y"].reshape(-1) for r in res.results]
    )  # [B] in batch order
    return out.reshape(B, O).astype(np.float32)



# revision 2
# speedup vs baseline: 1.1484x; 1.1484x over previous
"""Trainium2 Bass kernel for nn_LSTMModel (B=2048, T=512, I=1, H=64, O=1).

Strategy: pure data parallel over 8 NeuronCores (256 batch rows each).
Within a core, 2 staggered groups of 128 batch columns run the T=512
recurrence, software-pipelined against each other.

Per group and step, gates are computed with gate-pairs stacked on 128
partitions (stationary [f|g] and [o|i]), so ONE sigmoid activation over
[128, 256] covers all four gates at half the per-element cycles of a
64-partition layout.  All elementwise ops run on DVE in bf16 with
SBUF-resident operands (4x perf mode).  Matmuls are bf16 (1 cycle/row
vs fp32's 4).

Partition-base choreography (verifier requires equal bases for SBUF
input pairs; outputs may shift):
  f^ @0, g^ @64 (pair 1);  o^ @0, i^ @64 (pair 2)
  q = (g^@64 - 0.5) * i^@64      -> q@0
  r = f^@0 * c@0                 -> r@0
  c = q@0 + r@0                  -> c@0 (in place)
  tau = tanh(2*c)@0              -> tau@0
  h~ = (tau@0 * 0.5) * o^@0      -> state rows 0:64

Math (weights pre-scaled on host, h~ = h/2, c~ = c/2):
  g-gate preactivation doubled so sigmoid(2a) = (tanh(a)+1)/2 serves it
  in the same activation op; x_t and the bias ride the recurrent matmul
  as two extra K rows (x via small prefetched DMAs, ones row static).
"""

import numpy as np

B, T, I, H, O = 2048, 512, 1, 64, 1
NCORES = 8
BC = B // NCORES          # 256 batch rows per core
NGROUPS = 2
WG = BC // NGROUPS        # 128 batch rows per group
NB = 4                    # state ring buffers per group
K = H + 2                 # h rows + ones row + x row
XLEAD = 2                 # x prefetch distance (steps)

_CACHE = {}


def _build_program(T=T):
    import concourse.bacc as bacc
    import concourse.tile as tile
    from concourse import mybir

    f32 = mybir.dt.float32
    bf16 = mybir.dt.bfloat16
    AF = mybir.ActivationFunctionType
    OP = mybir.AluOpType

    nc = bacc.Bacc("TRN2", target_bir_lowering=False, debug=False)

    w1_d = nc.dram_tensor("w1", (K, 128), bf16, kind="ExternalInput").ap()
    w2_d = nc.dram_tensor("w2", (K, 128), bf16, kind="ExternalInput").ap()
    wout_d = nc.dram_tensor("wout", (K, 1), bf16, kind="ExternalInput").ap()
    xt_d = [
        nc.dram_tensor(f"xt{g}", (T, WG), bf16, kind="ExternalInput").ap()
        for g in range(NGROUPS)
    ]
    y_d = nc.dram_tensor("y", (NGROUPS, WG), f32, kind="ExternalOutput").ap()

    with tile.TileContext(nc) as tc:
        with (
            tc.tile_pool(name="consts", bufs=1) as wpool,
            tc.tile_pool(name="state", bufs=1) as spool,
            tc.tile_pool(name="gates", bufs=2) as gpool,
            tc.tile_pool(name="tmp", bufs=3) as tpool,
            tc.tile_pool(name="psum", bufs=2, space="PSUM") as pspool,
            tc.tile_pool(name="opsum", bufs=1, space="PSUM") as opspool,
        ):
            w1 = wpool.tile([K, 128], bf16, tag="w1")
            w2 = wpool.tile([K, 128], bf16, tag="w2")
            wo = wpool.tile([K, 1], bf16, tag="wo")
            nc.sync.dma_start(w1[:], w1_d[:])
            nc.sync.dma_start(w2[:], w2_d[:])
            nc.sync.dma_start(wo[:], wout_d[:])

            cst = []   # cell state per group [64, WG] bf16
            st = []    # state ring per group: NB x [K, WG] bf16
            for g in range(NGROUPS):
                c = spool.tile([H, WG], bf16, tag=f"c{g}")
                nc.vector.memset(c[:], 0.0)
                cst.append(c)
                bufs = []
                for b in range(NB):
                    s = spool.tile([K, WG], bf16, tag=f"s{g}_{b}")
                    nc.vector.memset(s[0:H, :], 0.0)
                    nc.vector.memset(s[H : H + 1, :], 1.0)
                    bufs.append(s)
                st.append(bufs)

            # pre-issue x DMAs for the first XLEAD steps
            for g in range(NGROUPS):
                for t0 in range(XLEAD):
                    nc.sync.dma_start(
                        st[g][t0 % NB][H + 1 : K, :], xt_d[g][t0 : t0 + 1, :]
                    )

            for t in range(T):
                for g in range(NGROUPS):
                    sb = st[g][t % NB]
                    c = cst[g]
                    # prefetch x for step t+XLEAD
                    tp = t + XLEAD
                    if tp < T:
                        nc.sync.dma_start(
                            st[g][tp % NB][H + 1 : K, :], xt_d[g][tp : tp + 1, :]
                        )
                    # two M=128 matmuls: [f|g] and [o|i] gate pairs
                    ps = pspool.tile([128, 2 * WG], f32, tag=f"ps{g}")
                    nc.tensor.matmul(
                        ps[:, 0:WG], w1[:], sb[:], start=True, stop=True
                    )
                    nc.tensor.matmul(
                        ps[:, WG : 2 * WG], w2[:], sb[:], start=True, stop=True
                    )
                    # one sigmoid for all 4 gates: [128, 2*WG]
                    gt = gpool.tile([128, 2 * WG], bf16, tag=f"g{g}")
                    nc.scalar.activation(gt[:], ps[:], AF.Sigmoid)
                    fh = gt[0:H, 0:WG]            # f^ @0
                    gh = gt[H:128, 0:WG]          # g^ @64
                    oh = gt[0:H, WG : 2 * WG]     # o^ @0
                    ih = gt[H:128, WG : 2 * WG]   # i^ @64
                    # q~ = (g^ - 0.5) * i^   (inputs @64 -> out @0)
                    q = tpool.tile([H, WG], bf16, tag=f"q{g}")
                    nc.vector.scalar_tensor_tensor(
                        q[:], gh, 0.5, ih, OP.subtract, OP.mult
                    )
                    # r = f^ * c~
                    r = tpool.tile([H, WG], bf16, tag=f"r{g}")
                    nc.vector.tensor_mul(r[:], fh, c[:])
                    # c~ = q~ + r (in place)
                    nc.vector.tensor_add(c[:], q[:], r[:])
                    # tau = tanh(2 c~) = tanh(c)
                    tch = tpool.tile([H, WG], bf16, tag=f"t{g}")
                    nc.scalar.activation(tch[:], c[:], AF.Tanh, scale=2.0)
                    # h~ = (tau * 0.5) * o^ -> next state buffer's h rows
                    nxt = st[g][(t + 1) % NB]
                    nc.vector.scalar_tensor_tensor(
                        nxt[0:H, :], tch[:], 0.5, oh, OP.mult, OP.mult
                    )

            for g in range(NGROUPS):
                pso = opspool.tile([1, WG], f32, tag="po")
                nc.tensor.matmul(
                    pso[:], wo[:], st[g][T % NB][:], start=True, stop=True
                )
                yt = tpool.tile([1, WG], f32, tag=f"y{g}")
                nc.vector.tensor_copy(yt[:], pso[:])
                nc.sync.dma_start(y_d[g : g + 1, :], yt[:])

    nc.compile()
    return nc


def _prep_weights(w_ih, w_hh, b_ih, b_hh, w_lin, b_lin):
    """Host-side pre-scaled stationary operands ([K, M] = lhsT layout)."""
    import ml_dtypes

    w_hh = np.asarray(w_hh, np.float32)
    w_ih = np.asarray(w_ih, np.float32)
    bias = np.asarray(b_ih, np.float32) + np.asarray(b_hh, np.float32)
    # per-gate-row scale: 1 for i,f,o; 2 for g (sigmoid(2a) trick)
    s = np.ones((4 * H, 1), np.float32)
    s[2 * H : 3 * H] = 2.0
    # state row layout: [h~ (0:64); ones (64); x (65)]; W_hh doubled for h~
    waug = np.concatenate(
        [2.0 * s * w_hh, s * bias[:, None], s * w_ih[:, :1]], axis=1
    )  # [4H, K]
    i_r = slice(0, H)
    f_r = slice(H, 2 * H)
    g_r = slice(2 * H, 3 * H)
    o_r = slice(3 * H, 4 * H)
    # lhsT col-blocks: w1 = [f | g], w2 = [o | i]
    w1 = np.ascontiguousarray(
        np.concatenate([waug[f_r], waug[g_r]], axis=0).T
    ).astype(ml_dtypes.bfloat16)  # [K, 128]
    w2 = np.ascontiguousarray(
        np.concatenate([waug[o_r], waug[i_r]], axis=0).T
    ).astype(ml_dtypes.bfloat16)  # [K, 128]
    wout = np.zeros((K, 1), np.float32)
    wout[0:H, 0] = 2.0 * np.asarray(w_lin, np.float32)[0]
    wout[H, 0] = float(np.asarray(b_lin, np.float32)[0])
    return w1, w2, wout.astype(ml_dtypes.bfloat16)


def _make_in_maps(x, w_ih, w_hh, b_ih, b_hh, w_lin, b_lin):
    import ml_dtypes

    w1, w2, wout = _prep_weights(w_ih, w_hh, b_ih, b_hh, w_lin, b_lin)
    x = np.asarray(x, np.float32).reshape(B, T)  # I == 1
    in_maps = []
    for core in range(NCORES):
        xc = x[core * BC : (core + 1) * BC]  # [BC, T]
        m = {"w1": w1, "w2": w2, "wout": wout}
        for g in range(NGROUPS):
            xg = xc[g * WG : (g + 1) * WG]  # [WG, T]
            m[f"xt{g}"] = np.ascontiguousarray(xg.T).astype(
                ml_dtypes.bfloat16
            )  # [T, WG]
        in_maps.append(m)
    return in_maps


def kernel(x, w_ih, w_hh, b_ih, b_hh, w_lin, b_lin):
    from concourse import bass_utils

    if "nc" not in _CACHE:
        _CACHE["nc"] = _build_program()
    nc = _CACHE["nc"]

    in_maps = _make_in_maps(x, w_ih, w_hh, b_ih, b_hh, w_lin, b_lin)
    res = bass_utils.run_bass_kernel_spmd(
        nc, in_maps, core_ids=list(range(NCORES))
    )
    out = np.concatenate(
        [r["y"].reshape(-1) for r in res.results]
    )  # [B] in batch order
    return out.reshape(B, O).astype(np.float32)


# revision 13
# speedup vs baseline: 1.2368x; 1.0770x over previous
"""Trainium2 Bass kernel for nn_LSTMModel (B=2048, T=512, I=1, H=64, O=1).

Strategy: pure data parallel over 8 NeuronCores (256 batch rows each).
Within a core, 3 staggered groups (86/85/85 batch columns) run the T=512
recurrence as software-pipelined independent chains.  The per-step
dependency cycle (MM -> sigmoid -> cell update -> tanh -> h -> MM) is
latency-bound, so group width is chosen to shorten each op on the cycle
while keeping total ACT-engine busy below the cycle latency.

Per group and step, gates are computed with gate-pairs stacked on 128
partitions (stationary [f|g] and [o|i]), so ONE sigmoid activation over
[128, 2*Wg] covers all four gates.  State/gates/weights are fp16
(1 matmul cycle/row vs fp32's 4; packed 2-byte = 2x DVE mode); the cell
state c -- the 512-step integrator -- stays fp32 for accuracy.

Partition-base choreography (verifier requires equal bases for SBUF
input pairs; outputs may shift):
  f^ @0, g^ @64 (pair 1);  o^ @0, i^ @64 (pair 2)
  q = (g^@64 - 0.5) * i^@64      -> q@0      (STT on DVE)
  r = f^@0 * c@0                 -> r@0      (TT on gpsimd, parallel with q)
  c = q@0 + r@0                  -> c@0      (TT, in place)
  tau = tanh(2*c)@0              -> tau@0    (ACT)
  h = tau@0 * o^@0               -> state rows 0:64  (TT)

Math (weights pre-scaled on host, c~ = c/2, h stored at full scale):
  g-gate preactivation doubled so sigmoid(2a) = (tanh(a)+1)/2 serves it
  inside the big sigmoid op; x_t and the bias ride the recurrent matmul
  as two extra K rows.  x rows are DMA'd two steps per transfer into a
  single ring tile, ~3 steps ahead (emitted after the matmuls that last
  read the target ring slots, so the WAR ordering is correct).

Anti-phase seeding: groups 1 and 2 get an artificial zero-valued
dependency on group 0's first sigmoid / cell-add outputs so the three
chains settle ~1/3 period apart instead of locksteppping.
"""

import numpy as np

B, T, I, H, O = 2048, 512, 1, 64, 1
NCORES = 8
BC = B // NCORES            # 256 batch rows per core
GROUPS = [86, 85, 85]       # batch columns per group (sum = BC)
NG = len(GROUPS)
NB = 4                      # state ring buffers per group (even, for x pairs)
K = H + 2                   # h rows + ones row + x row

_CACHE = {}


def _build_program(T=T):
    import concourse.bacc as bacc
    import concourse.tile as tile
    from concourse import mybir

    f32 = mybir.dt.float32
    bf16 = mybir.dt.float16   # 2-byte dtype for state/gates/weights
    AF = mybir.ActivationFunctionType
    OP = mybir.AluOpType

    nc = bacc.Bacc("TRN2", target_bir_lowering=False, debug=False)

    w1_d = nc.dram_tensor("w1", (K, 128), bf16, kind="ExternalInput").ap()
    w2_d = nc.dram_tensor("w2", (K, 128), bf16, kind="ExternalInput").ap()
    wout_d = nc.dram_tensor("wout", (K, 1), bf16, kind="ExternalInput").ap()
    xt_d = [
        nc.dram_tensor(f"xt{g}", (T // 2, 2 * wg), bf16, kind="ExternalInput").ap()
        for g, wg in enumerate(GROUPS)
    ]
    y_d = nc.dram_tensor("y", (1, BC), f32, kind="ExternalOutput").ap()

    with tile.TileContext(nc) as tc:
        with (
            tc.tile_pool(name="consts", bufs=1) as wpool,
            tc.tile_pool(name="state", bufs=1) as spool,
            tc.tile_pool(name="gates", bufs=2) as gpool,
            tc.tile_pool(name="tmp", bufs=3) as tpool,
            tc.tile_pool(name="psum", bufs=2, space="PSUM") as pspool,
            tc.tile_pool(name="opsum", bufs=1, space="PSUM") as opspool,
        ):
            w1 = wpool.tile([K, 128], bf16, tag="w1")
            w2 = wpool.tile([K, 128], bf16, tag="w2")
            wo = wpool.tile([K, 1], bf16, tag="wo")
            zz = wpool.tile([H, 128], bf16, tag="zz")   # zeros, for phase seeds
            nc.sync.dma_start(w1[:], w1_d[:])
            nc.sync.dma_start(w2[:], w2_d[:])
            nc.sync.dma_start(wo[:], wout_d[:])
            nc.vector.memset(zz[:], 0.0)

            cst = []    # cell state per group [H, Wg] fp32 (the integrator)
            srng = []   # state ring per group: one [K, NB*Wg] tile
            for g, wg in enumerate(GROUPS):
                c = spool.tile([H, wg], f32, tag=f"c{g}")
                nc.vector.memset(c[:], 0.0)
                cst.append(c)
                s = spool.tile([K, NB * wg], bf16, tag=f"s{g}")
                nc.vector.memset(s[0:H, :], 0.0)
                nc.vector.memset(s[H : H + 1, :], 1.0)
                srng.append(s)

            # pre-issue x pair-DMAs for steps 0..3 (ring slots 0..3)
            for g, wg in enumerate(GROUPS):
                for p in range(2):
                    nc.sync.dma_start(
                        srng[g][H + 1 : K, p * 2 * wg : (p + 1) * 2 * wg],
                        xt_d[g][p : p + 1, :],
                    )

            seed_src = [None, None]  # [gt of group0 step0, c of group0]

            for t in range(T):
                for g, wg in enumerate(GROUPS):
                    s = srng[g]
                    c = cst[g]
                    sb = s[:, (t % NB) * wg : (t % NB + 1) * wg]
                    # two M=128 matmuls: [f|g] and [o|i] gate pairs
                    ps = pspool.tile([128, 2 * wg], f32, tag=f"ps{g}")
                    nc.tensor.matmul(
                        ps[:, 0:wg], w1[:], sb, start=True, stop=True
                    )
                    nc.tensor.matmul(
                        ps[:, wg : 2 * wg], w2[:], sb, start=True, stop=True
                    )
                    # one sigmoid for all 4 gates: [128, 2*Wg]
                    gt = gpool.tile([128, 2 * wg], bf16, tag=f"g{g}")
                    nc.scalar.activation(gt[:], ps[:], AF.Sigmoid)
                    fh = gt[0:H, 0:wg]            # f^ @0
                    gh = gt[H:128, 0:wg]          # g^ @64
                    oh = gt[0:H, wg : 2 * wg]     # o^ @0
                    ih = gt[H:128, wg : 2 * wg]   # i^ @64
                    # q~ = (g^ - 0.5) * i^   (inputs @64 -> out @0)
                    q = tpool.tile([H, wg], f32, tag=f"q{g}")
                    nc.vector.scalar_tensor_tensor(
                        q[:], gh, 0.5, ih, OP.subtract, OP.mult
                    )
                    # r = f^ * c~  on gpsimd, concurrent with q on DVE
                    r = tpool.tile([H, wg], f32, tag=f"r{g}")
                    nc.gpsimd.tensor_mul(r[:], fh, c[:])
                    # c~ = q~ + r (in place)
                    nc.vector.tensor_add(c[:], q[:], r[:])
                    # tau = tanh(2 c~) = tanh(c)
                    tch = tpool.tile([H, wg], bf16, tag=f"t{g}")
                    nc.scalar.activation(tch[:], c[:], AF.Tanh, scale=2.0)
                    # h = tau * o^ -> next state buffer's h rows
                    nxt = s[0:H, ((t + 1) % NB) * wg : ((t + 1) % NB + 1) * wg]
                    nc.vector.tensor_mul(nxt, tch[:], oh)

                    # paired x prefetch: at odd t (after this step's MMs are
                    # emitted), fetch steps t+3, t+4 into the two ring slots
                    # just consumed by MM(t-1) and MM(t)
                    if t % 2 == 1 and t + 3 < T:
                        p = (t + 3) // 2
                        col = ((t + 3) % NB) * wg
                        nc.sync.dma_start(
                            s[H + 1 : K, col : col + 2 * wg],
                            xt_d[g][p : p + 1, :],
                        )

                    # anti-phase seeds, once, after group 0's step 0
                    if t == 0 and g == 0:
                        seed_src[0] = gt
                        seed_src[1] = c
                    if t == 0 and g < NG - 1:
                        wgn = GROUPS[g + 1]
                        src = seed_src[g][0:H, 0:wgn]
                        nc.vector.tensor_mul(
                            srng[g + 1][0:H, 0:wgn], src, zz[:, 0:wgn]
                        )

            col0 = 0
            for g, wg in enumerate(GROUPS):
                sb = srng[g][:, (T % NB) * wg : (T % NB + 1) * wg]
                pso = opspool.tile([1, wg], f32, tag="po")
                nc.tensor.matmul(pso[:], wo[:], sb, start=True, stop=True)
                yt = tpool.tile([1, wg], f32, tag=f"y{g}")
                nc.vector.tensor_copy(yt[:], pso[:])
                nc.sync.dma_start(y_d[0:1, col0 : col0 + wg], yt[:])
                col0 += wg

    nc.compile()
    return nc


def _prep_weights(w_ih, w_hh, b_ih, b_hh, w_lin, b_lin):
    """Host-side pre-scaled stationary operands ([K, M] = lhsT layout)."""
    import ml_dtypes

    w_hh = np.asarray(w_hh, np.float32)
    w_ih = np.asarray(w_ih, np.float32)
    bias = np.asarray(b_ih, np.float32) + np.asarray(b_hh, np.float32)
    # per-gate-row scale: 1 for i,f,o; 2 for g (sigmoid(2a) trick)
    s = np.ones((4 * H, 1), np.float32)
    s[2 * H : 3 * H] = 2.0
    # state rows: [h (0:64, FULL scale); ones (64); x (65)]
    waug = np.concatenate(
        [s * w_hh, s * bias[:, None], s * w_ih[:, :1]], axis=1
    )  # [4H, K]
    i_r = slice(0, H)
    f_r = slice(H, 2 * H)
    g_r = slice(2 * H, 3 * H)
    o_r = slice(3 * H, 4 * H)
    # lhsT col-blocks: w1 = [f | g], w2 = [o | i]
    w1 = np.ascontiguousarray(
        np.concatenate([waug[f_r], waug[g_r]], axis=0).T
    ).astype(np.float16)  # [K, 128]
    w2 = np.ascontiguousarray(
        np.concatenate([waug[o_r], waug[i_r]], axis=0).T
    ).astype(np.float16)  # [K, 128]
    wout = np.zeros((K, 1), np.float32)
    wout[0:H, 0] = np.asarray(w_lin, np.float32)[0]
    wout[H, 0] = float(np.asarray(b_lin, np.float32)[0])
    return w1, w2, wout.astype(np.float16)


def _make_in_maps(x, w_ih, w_hh, b_ih, b_hh, w_lin, b_lin):
    import ml_dtypes

    w1, w2, wout = _prep_weights(w_ih, w_hh, b_ih, b_hh, w_lin, b_lin)
    x = np.asarray(x, np.float32).reshape(B, T)  # I == 1
    in_maps = []
    for core in range(NCORES):
        xc = x[core * BC : (core + 1) * BC]  # [BC, T]
        m = {"w1": w1, "w2": w2, "wout": wout}
        g0 = 0
        for g, wg in enumerate(GROUPS):
            xg = xc[g0 : g0 + wg]  # [Wg, T]
            g0 += wg
            # [T, Wg] -> pairs of steps side by side: [T//2, 2*Wg]
            xt = np.ascontiguousarray(xg.T).reshape(T // 2, 2 * wg)
            m[f"xt{g}"] = xt.astype(np.float16)
        in_maps.append(m)
    return in_maps


def kernel(x, w_ih, w_hh, b_ih, b_hh, w_lin, b_lin):
    from concourse import bass_utils

    if "nc" not in _CACHE:
        _CACHE["nc"] = _build_program()
    nc = _CACHE["nc"]

    in_maps = _make_in_maps(x, w_ih, w_hh, b_ih, b_hh, w_lin, b_lin)
    res = bass_utils.run_bass_kernel_spmd(
        nc, in_maps, core_ids=list(range(NCORES))
    )
    out = np.concatenate(
        [r["y"].reshape(-1) for r in res.results]
    )  # [B] in batch order
    return out.reshape(B, O).astype(np.float32)
